# revision 3
# baseline (speedup 1.0000x reference)
"""Trainium2 Bass kernel for CustomScaledDotProductAttention, v2.

y = out_proj(softmax(q k^T / sqrt(D)) v), fused qkv proj.
x [2, 2048, 1024], H=16 heads, D=64.  8 cores: core = b*4 + g (batch x
head-group of 4 heads).  Host sums 4 out-proj partials per batch + b_out.

v2 redesign vs the 223us baseline (~213us measured; trace-driven):
  - the baseline spent 53us in a serial projection phase (ScalarE idle)
    and a 31us tail (DRAM-bounce stall + cold-clock out-proj).  v2 folds
    ALL projection work into the attention pipeline as per-m-step
    "extras": the exp stream starts ~22us in (engine boot + x DMA) and
    runs nearly gapless to the end.
  - block order: 4 pair-0 blocks first (B0 carries v-proj just-in-time +
    K01; B1-B4 carry the remaining qk projections), then 4 pair-1 blocks
    carrying the out-proj t-blocks.
  - x/w DMA in partition-major host layouts (4-16KB contiguous lines,
    x in quarters, wq split by j-group so the first stationary lands
    early).  Dependency-free dummy matmuls on a scratch tile pre-warm
    the PE HAM clock to 2.4GHz while the DMAs stream.
  - epilogue (per block): l rows reshaped by two 32-descriptor SBUF DMAs
    into [32,32], DVE reciprocal -> bf16, reshaped back to [2,512]; the
    1/l broadcast is a PE ones-matmul deferred into the NEXT block's
    slot 6 (never lets the in-order PE stream wait on a reshape DMA).
    No DRAM bounce anywhere.
  - y written per t-block (cast + DMA at evacuation): DMA descriptors
    enqueue when their wait-sem fires, so firing at evac time keeps the
    fat y descriptors from head-blocking the epilogue reshape DMAs in
    the queue FIFOs.  B7's last two y DMAs are deferred past the final
    lt reshape; paced dummy matmuls (gated on the chain's own
    intermediates) keep the PE warm through the final reciprocal chain.
  - v-proj drops the 323-wide padding: attn@v stationary slices use
    col offsets {0,65,130} at width 128 and {195} at width 65 (a 128-
    wide slice for the last head would put psum rows at base 63, which
    breaks the 32-aligned partition-base rule).
  - v-bias + ones column via a broadcast-rows tile on the DVE
    evacuation add (kills the 1-row bias matmuls).
  - ScalarE does nothing but exp (the attention pace-setter at
    ~1.08us per [128,2,512] EXP; PE is the overall limiter at ~182us
    busy).  fp8 and Schraudolph-exp offload were evaluated and
    rejected: fp8 breaks the 2e-2 gate on this data (errors do not
    average out in random-data attention), and DVE-exp (1.24us) is
    slower than the ACT exp it would replace in the slot chain.
"""

import numpy as np

import concourse.bass as bass
import concourse.mybir as mybir
import concourse.tile as tile
from concourse.bass_utils import run_bass_kernel_spmd

F32 = mybir.dt.float32
BF16 = mybir.dt.bfloat16

B, N, C, H, D = 2, 2048, 1024, 16, 64
SCALE = D ** -0.5          # 0.125
HPC = 4                    # heads per core
N_CORES = 8
NK = C // 128              # 8 contraction chunks of 128
NM = N // 128              # 16 m-chunks (key blocks)
NN = N // 512              # 4 n-chunks (query blocks of 512)
VW = HPC * (D + 1)         # 260: v columns + ones column per head
# attn@v stationary start col and width per (pair, hh).  The last head's
# slice is 65 wide (a 128-wide slice would need psum rows at base 63,
# which violates the 32-aligned partition-base rule downstream).
C0 = {(0, 0): (0, 128), (0, 1): (65, 128), (1, 0): (130, 128),
      (1, 1): (195, 65)}
ORDER = [(0, 0), (0, 1), (0, 2), (0, 3), (1, 0), (1, 1), (1, 2), (1, 3)]


def _emit(tc, nc, ten):
    PS = bass.MemorySpace.PSUM
    EXP = mybir.ActivationFunctionType.Exp

    with (
        nc.allow_low_precision(reason="bf16 operands; f32 psum"),
        tc.tile_pool(name="persist", bufs=1) as pp,
        tc.tile_pool(name="qk", bufs=1) as qkp,
        tc.tile_pool(name="vp", bufs=1) as vp,
        tc.tile_pool(name="at", bufs=1) as atp,
    ):
        # ---- persistent SBUF tiles ----
        xt = pp.tile([128, 4, NK, 512], BF16, tag="xt", name="xt")
        wq = pp.tile([128, 4, NK, 128], BF16, tag="wq", name="wq")
        wv = pp.tile([128, NK, VW], BF16, tag="wv", name="wv")
        wo = pp.tile([128, 2, 1024], BF16, tag="wo", name="wo")
        bqs = pp.tile([128, 4], F32, tag="bqs", name="bqs")
        bvb = pp.tile([128, VW], BF16, tag="bvb", name="bvb")
        ones2 = pp.tile([2, 128], BF16, tag="ones2", name="ones2")
        QK = [qkp.tile([128, N], BF16, tag=f"qk{j}", name=f"qk{j}")
              for j in range(4)]
        Vb = vp.tile([128, NM, VW], BF16, tag="vb", name="vb")
        AT = [atp.tile([128, N], BF16, tag=f"at{p}", name=f"at{p}")
              for p in range(2)]

        # ---- input DMAs: x quarter 0 + wq lead (they gate the first
        # matmuls); everything else queues behind them ----
        nc.sync.dma_start(out=xt[:, 0, :, :], in_=ten["x"][:, 0, :, :])
        nc.gpsimd.dma_start(out=wq[:, 1, :, :], in_=ten["wq"][:, 1, :, :])
        nc.gpsimd.dma_start(out=wq[:, 0, :, :], in_=ten["wq"][:, 0, :, :])
        nc.gpsimd.dma_start(out=bqs, in_=ten["bq"][:, :])
        nc.sync.dma_start(out=wv, in_=ten["wv"][:, :, :])
        nc.sync.dma_start(out=bvb, in_=ten["bvb"][:, :])
        for qd in range(1, 4):
            nc.sync.dma_start(out=xt[:, qd, :, :],
                              in_=ten["x"][:, qd, :, :])
        nc.gpsimd.dma_start(out=wq[:, 2, :, :], in_=ten["wq"][:, 2, :, :])
        nc.gpsimd.dma_start(out=wq[:, 3, :, :], in_=ten["wq"][:, 3, :, :])
        nc.gpsimd.dma_start(out=ones2, in_=ten["ones2"][:, :])
        nc.gpsimd.dma_start(out=wo, in_=ten["wo"][:, :, :])

        # x column block n lives at xt[:, n, c, :]  (n is the DMA quarter)
        def xmov(cc, n):
            return xt[:, n, cc, :]

        def emit_qk_mm(pq, j, n, cc):
            nc.tensor.matmul(pq, wq[:, j, cc, :],
                             xmov(cc, n), start=(cc == 0), stop=(cc == NK - 1))

        with (
            tc.tile_pool(name="sc", bufs=2, space=PS) as scp,
            tc.tile_pool(name="ot", bufs=1, space=PS) as otp,
            tc.tile_pool(name="et", bufs=3) as etp,
            tc.tile_pool(name="orw", bufs=4) as orp,
            tc.tile_pool(name="lv", bufs=4) as lvp,
            tc.tile_pool(name="bcs", bufs=4) as bcp,
            tc.tile_pool(name="ysb", bufs=3) as ysbp,
        ):
            # ============ work-unit emitters (extras) ============
            state = {"pq": {}, "psv": {}, "yp": {}, "ysg": {}}

            def mk_qk(j, n, cclist, ep):
                def go():
                    key = (j, n)
                    if key not in state["pq"]:
                        state["pq"][key] = ep.tile(
                            [128, 512], F32, tag="e", name=f"pq{j}{n}")
                    for cc in cclist:
                        emit_qk_mm(state["pq"][key], j, n, cc)
                return go

            def mk_qk_evac(j, n):
                def go():
                    pq = state["pq"].pop((j, n))
                    nc.vector.tensor_scalar_add(
                        QK[j][:, n * 512:(n + 1) * 512], pq, bqs[:, j:j + 1])
                return go

            def mk_v(m, half, ep):
                def go():
                    if m not in state["psv"]:
                        state["psv"][m] = ep.tile(
                            [128, 512], F32, tag="e", name=f"pv{m}")
                    psv = state["psv"][m]
                    ccs = range(0, 4) if half == 0 else range(4, NK)
                    for cc in ccs:
                        nc.tensor.matmul(
                            psv[:, 0:VW],
                            xt[:, m // 4, cc, (m % 4) * 128:(m % 4) * 128 + 128],
                            wv[:, cc, :],
                            start=(cc == 0), stop=(cc == NK - 1))
                    if half == 1:
                        psv = state["psv"].pop(m)
                        nc.vector.tensor_add(Vb[:, m, :], psv[:, 0:VW], bvb)
                return go

            def mk_out(t, ic, ep, tag="yp"):
                def go():
                    if t not in state["yp"]:
                        state["yp"][t] = ep.tile(
                            [128, 2, 512], F32, tag=tag, name=f"yp{t}")
                    for oc in range(2):
                        nc.tensor.matmul(
                            state["yp"][t][:, oc, :],
                            AT[ic][:, t * 128:(t + 1) * 128],
                            wo[:, ic, oc * 512:(oc + 1) * 512],
                            start=(ic == 0), stop=(ic == 1))
                return go

            def mk_out_evac(t, defer=False):
                # per-t cast + DMA fired immediately: descriptors enqueue
                # when their wait-sem fires, so y data drains during the
                # block instead of head-blocking the epilogue's reshape
                # DMAs in the queue FIFOs at the tail.  defer=True delays
                # the DMA (not the cast) past the last epilogue's reshapes.
                def go():
                    yp = state["yp"].pop(t)
                    ysb = ysbp.tile([128, 2, 512], BF16, tag="ysb",
                                    name=f"ysb{t}")
                    nc.vector.tensor_copy(ysb, yp)
                    if defer:
                        state.setdefault("late", []).append(
                            lambda: nc.gpsimd.dma_start(
                                out=ten["y"][:, t, :], in_=ysb))
                    else:
                        nc.gpsimd.dma_start(out=ten["y"][:, t, :], in_=ysb)
                return go

            # ============ attention block machinery ============
            def emit_sp(p, n, m):
                Qt, Kt = QK[2 * p], QK[2 * p + 1]
                ncol = slice(n * 512, (n + 1) * 512)
                sp = scp.tile([128, 2, 512], F32, tag="sp", name="sp")
                nc.tensor.matmul(
                    sp[:, 0, :], Kt[0:64, m * 128:(m + 1) * 128],
                    Qt[0:64, ncol],
                    start=True, stop=True, tile_position=(0, 0))
                nc.tensor.matmul(
                    sp[:, 1, :], Kt[64:128, m * 128:(m + 1) * 128],
                    Qt[64:128, ncol],
                    start=True, stop=True, tile_position=(64, 0))
                return sp

            def attention_block(p, n, sp0, next_pn, extras, post=()):
                ncol = slice(n * 512, (n + 1) * 512)
                ot = otp.tile([128, 2, 512], F32, tag="ot", name="ot")

                sp_cur, nxt_sp0 = sp0, None
                for m in range(NM):
                    e = etp.tile([128, 2, 512], BF16, tag="et", name="et")
                    nc.scalar.activation(e, sp_cur, EXP, scale=SCALE)
                    if m + 1 < NM:
                        sp_nxt = emit_sp(p, n, m + 1)
                    elif next_pn is not None:
                        nxt_sp0 = emit_sp(next_pn[0], next_pn[1], 0)
                    for hh in range(2):
                        c0, w = C0[(p, hh)]
                        nc.tensor.matmul(
                            ot[0:w, hh, :], Vb[:, m, c0:c0 + w], e[:, hh, :],
                            start=(m == 0), stop=(m == NM - 1))
                    for fn in extras[m]:
                        fn()
                    if m + 1 < NM:
                        sp_cur = sp_nxt
                for fn in post:
                    fn()

                # epilogue part 1 (no PE instructions: the PE stream must
                # not block on the reshape DMAs): evacuate ot, reciprocal
                # of l via a 32x32 layout (32 descriptors per reshape DMA)
                orw = []
                for hh in range(2):
                    o = orp.tile([65, 512], F32, tag="orw", name="orw")
                    nc.vector.tensor_copy(o, ot[0:65, hh, :])
                    orw.append(o)
                lt = lvp.tile([32, 32], F32, tag="lv", name="lt")
                nc.sync.dma_start(out=lt[0:16, :], in_=orw[0][64:65, :])
                nc.sync.dma_start(out=lt[16:32, :], in_=orw[1][64:65, :])
                linvb = lvp.tile([32, 32], BF16, tag="lvb", name="livb")
                nc.vector.reciprocal(linvb, lt)
                lrow = lvp.tile([2, 512], BF16, tag="lrow", name="lrow")
                nc.sync.dma_start(out=lrow, in_=linvb)
                return nxt_sp0, (p, n, orw, linvb, lrow)

            def finish_epilogue(pend, bc_ap):
                # part 2, scheduled a few slots into the NEXT block so the
                # PE ones-matmul broadcast never waits on the lrow DMA
                p, n, orw, _, lrow = pend
                ncol = slice(n * 512, (n + 1) * 512)
                nc.tensor.matmul(bc_ap, ones2, lrow, start=True, stop=True)
                for hh in range(2):
                    nc.vector.tensor_mul(
                        AT[p][hh * 64:(hh + 1) * 64, ncol],
                        orw[hh][0:64, :], bc_ap[hh * 64:(hh + 1) * 64, :])

            # ============ schedule ============
            with tc.tile_pool(name="ep", bufs=2, space=PS) as ep:
                # --- HAM pre-warm: dependency-free dummy matmuls on an
                # uninitialized scratch tile run while the x/w DMAs stream,
                # so the real pre-B0 projections start at the 2.4 GHz warm
                # clock instead of 1.2 GHz cold ---
                scratch = pp.tile([128, 512], BF16, tag="scr", name="scr")
                nc.vector.memset(scratch, 0.0)
                pwarm = ep.tile([128, 512], F32, tag="e", name="pwarm")
                for w in range(26):
                    nc.tensor.matmul(pwarm, scratch[:, 0:128], scratch,
                                     start=True, stop=True)
                # --- pre-B0: K01 n0, Q01 n0, V m0-1 ---
                pk = ep.tile([128, 512], F32, tag="e", name="pk0")
                for cc in range(NK):
                    emit_qk_mm(pk, 1, 0, cc)
                nc.vector.tensor_scalar_add(QK[1][:, 0:512], pk, bqs[:, 1:2])
                state["pq"][(0, 0)] = ep.tile([128, 512], F32, tag="e",
                                              name="pq00")
                for cc in range(NK):
                    emit_qk_mm(state["pq"][(0, 0)], 0, 0, cc)
                mk_qk_evac(0, 0)()
                mk_v(0, 0, ep)()
                mk_v(0, 1, ep)()
                mk_v(1, 0, ep)()
                mk_v(1, 1, ep)()

                # --- extras tables for B0-B4 (projection work) ---
                # K01-n(g) feeds emit_sp(m=4g) in slot 4g-1, so each K group
                # must evacuate in slot 4g-2 at the latest (PE instructions
                # execute in order; a score matmul emitted before the last
                # chunk matmul of the K group it reads would deadlock).
                ex = {bi: [[] for _ in range(NM)] for bi in range(8)}
                # B0: v m2..15 just-in-time + K01 n1-3 + Q01 n1
                for m in range(2, NM):
                    s = m - 2
                    ex[0][s].append(mk_v(m, 0, ep))
                    ex[0][s].append(mk_v(m, 1, ep))
                ex[0][0].append(mk_qk(1, 1, [0, 1, 2], ep))
                ex[0][1].append(mk_qk(1, 1, [3, 4, 5], ep))
                ex[0][2].append(mk_qk(1, 1, [6, 7], ep))
                ex[0][2].append(mk_qk_evac(1, 1))
                ex[0][3].append(mk_qk(1, 2, [0, 1], ep))
                ex[0][4].append(mk_qk(1, 2, [2, 3], ep))
                ex[0][5].append(mk_qk(1, 2, [4, 5], ep))
                ex[0][6].append(mk_qk(1, 2, [6, 7], ep))
                ex[0][6].append(mk_qk_evac(1, 2))
                ex[0][7].append(mk_qk(1, 3, [0, 1], ep))
                ex[0][8].append(mk_qk(1, 3, [2, 3], ep))
                ex[0][9].append(mk_qk(1, 3, [4, 5], ep))
                ex[0][10].append(mk_qk(1, 3, [6, 7], ep))
                ex[0][10].append(mk_qk_evac(1, 3))
                ex[0][12].append(mk_qk(0, 1, [0, 1, 2], ep))
                ex[0][13].append(mk_qk(0, 1, [3, 4, 5], ep))
                ex[0][14].append(mk_qk(0, 1, [6, 7], ep))
                ex[0][14].append(mk_qk_evac(0, 1))
                # B1: K23 n0-1, Q01 n2
                for gi, (j, n) in enumerate(((3, 0), (3, 1))):
                    for a in range(4):
                        ex[1][4 * gi + a].append(
                            mk_qk(j, n, [2 * a, 2 * a + 1], ep))
                    ex[1][4 * gi + 3].append(mk_qk_evac(j, n))
                for a in range(4):
                    ex[1][8 + a].append(mk_qk(0, 2, [2 * a, 2 * a + 1], ep))
                ex[1][11].append(mk_qk_evac(0, 2))
                # B2: K23 n2-3, Q01 n3
                for gi, (j, n) in enumerate(((3, 2), (3, 3))):
                    for a in range(4):
                        ex[2][4 * gi + a].append(
                            mk_qk(j, n, [2 * a, 2 * a + 1], ep))
                    ex[2][4 * gi + 3].append(mk_qk_evac(j, n))
                for a in range(4):
                    ex[2][8 + a].append(mk_qk(0, 3, [2 * a, 2 * a + 1], ep))
                ex[2][11].append(mk_qk_evac(0, 3))
                # B3: Q23 n0, n1
                for gi, n in enumerate((0, 1)):
                    for a in range(4):
                        ex[3][4 * gi + a].append(
                            mk_qk(2, n, [2 * a, 2 * a + 1], ep))
                    ex[3][4 * gi + 3].append(mk_qk_evac(2, n))
                # B4: Q23 n2, n3
                for gi, n in enumerate((2, 3)):
                    for a in range(4):
                        ex[4][4 * gi + a].append(
                            mk_qk(2, n, [2 * a, 2 * a + 1], ep))
                    ex[4][4 * gi + 3].append(mk_qk_evac(2, n))

                # epilogue part 2 of block bi-1 runs in block bi's slot 3
                def mk_fin_ep(ep_):
                    def go():
                        pend = state.pop("pend")
                        bc = ep_.tile([128, 512], F32, tag="e", name="bc")
                        finish_epilogue(pend, bc)
                    return go

                # --- run B0..B4 ---
                sp0 = emit_sp(0, 0, 0)
                for bi in range(5):
                    p, n = ORDER[bi]
                    if "pend" in state:
                        ex[bi][6].append(mk_fin_ep(ep))
                    sp0, state["pend"] = attention_block(
                        p, n, sp0, ORDER[bi + 1], ex[bi])

            # --- B5..B7 with out-proj extras in a fresh psum pool ---
            with tc.tile_pool(name="ep2", bufs=1, space=PS) as ep2:
                def mk_fin_ep2():
                    def go():
                        pend = state.pop("pend")
                        bc = ep2.tile([128, 2, 512], F32, tag="yp",
                                      name="bc")
                        finish_epilogue(pend, bc[:, 0, :])
                    return go

                for bi in range(5, 8):
                    p, n = ORDER[bi]
                    tb = 4 * (bi - 5)      # t-group for AT pair ready so far
                    exb = [[] for _ in range(NM)]
                    exb[6].append(mk_fin_ep2())
                    for k in range(4):
                        t = tb + k
                        s0 = 8 + 2 * k
                        exb[s0].append(mk_out(t, 0, ep2))
                        exb[s0 + 1].append(mk_out(t, 1, ep2))
                        exb[s0 + 1].append(
                            mk_out_evac(t, defer=(bi == 7 and k >= 2)))
                    post = []
                    if bi == 7:
                        # t12.ic0: PE work that can run while the last
                        # epilogue's reciprocal chain resolves
                        post.append(mk_out(12, 0, ep2))
                    nxt = ORDER[bi + 1] if bi + 1 < 8 else None
                    sp0, state["pend"] = attention_block(p, n, sp0, nxt,
                                                         exb, post=post)

                # --- tail: finish B7's epilogue with HAM warm-keep dummy
                # matmuls paced by the chain's own intermediates, then
                # t12-15.  y DMAs go on the sync engine: its stream already
                # carries the lt/lrow triggers, so the fat y descriptors
                # enqueue after them.
                pend = state.pop("pend")
                _, _, orw7, linvb7, lrow7 = pend
                warm = scp.tile([128, 2, 512], F32, tag="sp", name="warm")
                bc7 = scp.tile([128, 2, 512], F32, tag="sp", name="bc7")
                for w in range(2):
                    nc.tensor.matmul(warm[:, 0, :], Vb[:, w, 0:128],
                                     QK[0][:, 0:512], start=True, stop=True)
                for w in range(2):      # gated on the ot evacuation (f32)
                    nc.tensor.matmul(warm[:, 1, :],
                                     orw7[w][0:64, 0:128], orw7[w][0:64, :],
                                     start=True, stop=True)
                # t13 ic0 only needs AT[0]: real work inside the warm chain
                mk_out(13, 0, scp, tag="sp")()
                for w in range(5):      # gated on the reciprocal's output;
                    nc.tensor.matmul(  # scribbles on bc7's unused half
                        bc7[0:32, 1, :], linvb7,
                        scratch[0:32, :], start=True, stop=True)
                finish_epilogue(pend, bc7[:, 0, :])
                for fn in state.pop("late", []):
                    fn()
                mk_out(12, 1, ep2)()
                mk_out(13, 1, scp, tag="sp")()
                mk_out_evac(12)()
                mk_out_evac(13)()
                mk_out(14, 0, ep2)()
                mk_out(14, 1, ep2)()
                mk_out(15, 0, scp, tag="sp")()
                mk_out_evac(14)()
                mk_out(15, 1, scp, tag="sp")()
                mk_out_evac(15)()


def _split_multi_waits(nc):
    """Hoist all-but-one sem wait from instructions onto standalone
    EventSemaphore instructions (single sync-wait slot per instruction)."""
    import bass_rust
    nop_id = [0]
    for fn in nc.m.functions:
        for blk in fn.blocks:
            insts = blk.instructions
            out = []
            changed = False
            for ins in insts:
                si = ins.sync_info
                is_evsem = isinstance(ins, mybir.InstEventSemaphore)
                if (si is not None and si.on_wait is not None
                        and len(si.on_wait) > 1 and not is_evsem):
                    waits = list(si.on_wait)
                    for w in waits[:-1]:
                        ev = mybir.InstEventSemaphore(
                            name=f"waitev_{nop_id[0]}", engine=ins.engine)
                        nop_id[0] += 1
                        ev.sync_info = bass_rust.SyncInfo(
                            on_wait=[w], on_update=[])
                        out.append(ev)
                    ins.sync_info = bass_rust.SyncInfo(
                        on_wait=[waits[-1]],
                        on_update=list(si.on_update or []))
                    changed = True
                out.append(ins)
            if changed:
                blk.instructions = out


def build_bass(split_waits=True):
    nc = bass.Bass()
    ten = {
        "x": nc.dram_tensor("x", [128, 4, NK, 512], BF16,
                            kind="ExternalInput"),
        "wq": nc.dram_tensor("wq", [128, 4, NK, 128], BF16,
                             kind="ExternalInput"),
        "bq": nc.dram_tensor("bq", [128, 4], F32, kind="ExternalInput"),
        "wv": nc.dram_tensor("wv", [128, NK, VW], BF16,
                             kind="ExternalInput"),
        "bvb": nc.dram_tensor("bvb", [128, VW], BF16, kind="ExternalInput"),
        "wo": nc.dram_tensor("wo", [128, 2, 1024], BF16,
                             kind="ExternalInput"),
        "ones2": nc.dram_tensor("ones2", [2, 128], BF16,
                                kind="ExternalInput"),
        "y": nc.dram_tensor("y", [128, NM, C], BF16, kind="ExternalOutput"),
        "lb": nc.dram_tensor("lb", [16, 512], F32),
    }
    with tile.TileContext(nc) as tc:
        _emit(tc, nc, ten)
    if split_waits:
        _split_multi_waits(nc)
    return nc


def prep_core_inputs(x, w_qkv, b_qkv, w_out, core):
    import ml_dtypes
    bf = ml_dtypes.bfloat16
    b, g = divmod(core, HPC)
    heads = [HPC * g + i for i in range(HPC)]
    f = np.float32
    h0, h1, h2, h3 = heads

    def q_rows(h):
        return w_qkv[h * D:(h + 1) * D]

    def k_rows(h):
        return w_qkv[C + h * D:C + (h + 1) * D]

    def v_rows(h):
        return w_qkv[2 * C + h * D:2 * C + (h + 1) * D]

    # x: [128, quarter, c, 512]; x4[p, qd, cc, col] = x[b][qd*512+col, cc*128+p]
    xT = np.ascontiguousarray(x[b].T, f)                   # [C, N]
    x4 = xT.reshape(NK, 128, 4, 512).transpose(1, 2, 0, 3)

    # wq: [128, c, 512]; wq4[p, cc, jcol] = Wqk_cat[jcol, cc*128+p]
    wqk_rows = np.concatenate([
        q_rows(h0), q_rows(h1), k_rows(h0), k_rows(h1),
        q_rows(h2), q_rows(h3), k_rows(h2), k_rows(h3)], 0)   # [512, C]
    # wqj[p, j, c, col] = wqk_rows[j*128+col, c*128+p]
    wq4 = wqk_rows.reshape(4, 128, NK, 128).transpose(3, 0, 2, 1)
    bqk = np.concatenate([
        b_qkv[h0 * D:(h0 + 1) * D], b_qkv[h1 * D:(h1 + 1) * D],
        b_qkv[C + h0 * D:C + (h0 + 1) * D],
        b_qkv[C + h1 * D:C + (h1 + 1) * D],
        b_qkv[h2 * D:(h2 + 1) * D], b_qkv[h3 * D:(h3 + 1) * D],
        b_qkv[C + h2 * D:C + (h2 + 1) * D],
        b_qkv[C + h3 * D:C + (h3 + 1) * D]], 0)               # [512]

    # wv: [128, c, 260]; v cols per head i at 65i..65i+63, ones col zeroed
    wv_cat = np.zeros((C, VW), f)
    bvb = np.zeros((VW,), f)
    for i, h in enumerate(heads):
        wv_cat[:, i * (D + 1):i * (D + 1) + D] = v_rows(h).T
        bvb[i * (D + 1):i * (D + 1) + D] = \
            b_qkv[2 * C + h * D:2 * C + (h + 1) * D]
        bvb[i * (D + 1) + D] = 1.0
    wv4 = wv_cat.reshape(NK, 128, VW).transpose(1, 0, 2)
    bvb_t = np.broadcast_to(bvb, (128, VW))

    woa = np.concatenate([w_out[:, h * D:(h + 1) * D].T for h in heads], 0)
    wo4 = woa.reshape(2, 128, 1024).transpose(1, 0, 2)

    ones2 = np.zeros((2, 128), f)
    ones2[0, 0:64] = 1.0
    ones2[1, 64:128] = 1.0

    return {
        "x": np.ascontiguousarray(x4).astype(bf),
        "wq": np.ascontiguousarray(wq4).astype(bf),
        "bq": np.ascontiguousarray(bqk.reshape(4, 128).T, f),
        "wv": np.ascontiguousarray(wv4).astype(bf),
        "bvb": np.ascontiguousarray(bvb_t).astype(bf),
        "wo": np.ascontiguousarray(wo4).astype(bf),
        "ones2": np.ascontiguousarray(ones2).astype(bf),
    }


def assemble_output(partials, b_out):
    y = np.empty((B, N, C), np.float32)
    for b in range(B):
        acc = partials[HPC * b].astype(np.float32)
        for g in range(1, HPC):
            acc = acc + partials[HPC * b + g].astype(np.float32)
        y[b] = acc + b_out.astype(np.float32)
    return y


_NC_CACHE = {}


def run(inputs, trace=False):
    x = np.asarray(inputs["x"], np.float32)
    w_qkv = np.asarray(inputs["w_qkv"], np.float32)
    b_qkv = np.asarray(inputs["b_qkv"], np.float32)
    w_out = np.asarray(inputs["w_out"], np.float32)
    b_out = np.asarray(inputs["b_out"], np.float32)

    if "nc" not in _NC_CACHE:
        _NC_CACHE["nc"] = build_bass()
    nc = _NC_CACHE["nc"]

    in_maps = [prep_core_inputs(x, w_qkv, b_qkv, w_out, core)
               for core in range(N_CORES)]
    res = run_bass_kernel_spmd(nc, in_maps, list(range(N_CORES)),
                               trace=trace)
    partials = [np.asarray(res.results[i]["y"])
                .transpose(1, 0, 2).reshape(N, C) for i in range(N_CORES)]
    return assemble_output(partials, b_out), res.exec_time_ns


def kernel(**inputs):
    y, _ = run(inputs, trace=False)
    return y
